# revision 3
# baseline (speedup 1.0000x reference)
"""FourierLayer TRN2 kernel: per-core DFT -> top-6 mask -> sparse inverse DFT.

Contract: kernel(input_tensor=(8,2048,512) f32) -> (8,2048,512) f32.
Each of the 8 NeuronCores processes one batch element (data-parallel over
batch; no cross-core communication).

Per-core pipeline (all big matmuls bf16 hi/lo split, fp32 PSUM accumulation):
  Re[k,d] = sum_t cos(2pi k t/T) x[t,d]      k = 1..1024 (k=1024 zero-padded)
  Im[k,d] = sum_t -sin(2pi k t/T) x[t,d]
  R2 = 2*Re, I2 = 2*Im  (x2 folded into PSUM eviction scale)
  mag = R2^2 + I2^2  (ordering-equivalent to |rfft|)
  theta[d] = 6th largest mag over k  (PE transpose -> vector.max top-8)
  mask = mag >= theta  (exactly the top-6 per channel; ties a.s. absent)
  out[t,d] = sum_k mask*R2*cos(2pi k t/T) + mask*I2*(-sin(2pi k t/T))

Raw bass with manual semaphores: this toolchain's walrus rejects instructions
carrying >2 sync commands, which rules out TileContext auto-sync (its kernel
tail drain waits on every proc lane). All DMAs go through gpsimd/SWDGE: each
128-partition transfer increments the DMA semaphore once per SDMA engine
(16 total), so a cumulative wait value is reached only when every transfer
counted in it has fully completed - cumulative thresholds are sound.
DMAs are coarsened (hi|lo matrices concatenated host-side, 4-chunk
super-loads) to amortize the ~2us per-dma_start SWDGE emission cost.
"""

from contextlib import ExitStack

import numpy as np
import ml_dtypes

import concourse.bass as bass
import concourse.mybir as mybir

BF16 = mybir.dt.bfloat16
F32 = mybir.dt.float32
AF = mybir.ActivationFunctionType
ALU = mybir.AluOpType

T = 2048          # time length
D = 512           # channels
KF = 1024         # padded frequency count (col j <-> k=j+1; col KF-1 zeroed)
NTC = T // 128    # 16 time chunks
NKC = KF // 128   # 8 freq chunks
NDC = D // 128    # 4 channel chunks
TOPK = 6
SC = 4            # forward super-chunk: tc-chunks per DMA
NSC = 2 * NTC // SC   # 8 forward super-chunks (2 components x 4)
RSI = 2           # inverse stream ring slots

# ---- semaphore schedules (cumulative values, 16 per DMA) ----
# s_dma units of 16, gpsimd program order: x 2 (->2); fwd supers 8 (S done at
#   3+S; ->10); inv0,inv1 (11,12); theta row DMAs (13..16); then interleaved
#   inv_j (j>=2) at 13+2j and out_i at 18+2i; total 46 units = 736.
# s_pe:  fwd groups (comp,tc) 1..32; mag transposes 33..64; ones-bcast 65;
#        inverse groups 66..81
# s_act: Re evicts 1..8; Im evicts 9..16; transpose copies 17..48; thb 49;
#        r2h/i2h casts 50..65; out evicts 66..81
# s_dve: mag 1..8; max8+th4 9..12; mask groups 13..20
# s_pool: ones 1; identity 2


def build_kernel(nc: bass.Bass):
    xh = nc.dram_tensor("xh", (T, D), BF16, kind="ExternalInput")
    xl = nc.dram_tensor("xl", (T, D), BF16, kind="ExternalInput")
    # forward DFT matrices, hi|lo concatenated along columns: [t, 2*KF]
    cf = nc.dram_tensor("cf", (T, 2 * KF), BF16, kind="ExternalInput")
    sf = nc.dram_tensor("sf", (T, 2 * KF), BF16, kind="ExternalInput")
    # inverse blocks per t-chunk: [tc, p, 4*KF] = [cih | cil | sih | sil],
    # where cih[tc, p, kc*128+u] = C[kc*128+p, tc*128+u] etc.
    iv = nc.dram_tensor("iv", (NTC, 128, 4 * KF), BF16, kind="ExternalInput")
    out = nc.dram_tensor("out", (T, D), F32, kind="ExternalOutput")

    with ExitStack() as ctx:
        def sb(name, shape, dtype):
            return ctx.enter_context(nc.sbuf_tensor(name, shape, dtype))

        xh_sb = sb("xh_sb", [128, NTC * D], BF16)
        xl_sb = sb("xl_sb", [128, NTC * D], BF16)
        # forward ring: 2 super-slots x (SC tc-chunks x 2KF hi|lo cols)
        cf_sb = sb("cf_sb", [128, 2 * SC * 2 * KF], BF16)
        # inverse ring: RSI slots x 4*KF
        iv_sb = sb("iv_sb", [128, RSI * 4 * KF], BF16)
        r2 = sb("r2", [128, NKC * D], F32)
        i2 = sb("i2", [128, NKC * D], F32)
        r2h = sb("r2h", [128, NKC * D], BF16)
        i2h = sb("i2h", [128, NKC * D], BF16)
        mag = sb("mag", [128, NKC * D], F32)
        mag_t = sb("mag_t", [128, NDC * KF], F32)
        m8 = sb("m8", [128, NDC * 8], F32)
        trows = [sb(f"trow{i}", [1, 128], F32) for i in range(NDC)]
        thb = sb("thb", [128, D], F32)
        ones = sb("ones", [1, 128], F32)
        ident = sb("ident", [128, 128], F32)
        msk = sb("msk", [128, D], F32)
        sqt = sb("sqt", [128, D], F32)
        ot_sb = sb("ot_sb", [128, 2 * D], F32)
        banks = [ctx.enter_context(nc.psum_tensor(f"pb{i}", [128, D], F32))
                 for i in range(8)]
        pb5 = banks[5]
        s_dma = ctx.enter_context(nc.semaphore())
        s_pe = ctx.enter_context(nc.semaphore())
        s_act = ctx.enter_context(nc.semaphore())
        s_dve = ctx.enter_context(nc.semaphore())
        s_pool = ctx.enter_context(nc.semaphore())
        block = ctx.enter_context(nc.Block())

        @block.gpsimd
        def _(gpsimd):
            # constants
            gpsimd.memset(ones[:], 1.0).then_inc(s_pool, 1)
            gpsimd.memset(ident[:], 0.0)
            gpsimd.drain()
            nc.gpsimd.affine_select(
                out=ident[:], in_=ident[:],
                compare_op=ALU.not_equal, fill=1.0, base=0,
                pattern=[[-1, 128]], channel_multiplier=1,
            ).then_inc(s_pool, 1)
            # x loads: single DMA each, (tc p) d -> p (tc d)
            gpsimd.dma_start(
                xh_sb[:, :],
                xh[:].rearrange("(a p) d -> p a d", p=128)).then_inc(s_dma, 16)
            gpsimd.dma_start(
                xl_sb[:, :],
                xl[:].rearrange("(a p) d -> p a d", p=128)).then_inc(s_dma, 16)
            # forward super-chunks: S = comp*2+G over (cf, sf)
            W = 2 * KF
            for S in range(NSC):
                comp, G = divmod(S, NSC // 2)
                src = (cf, sf)[comp]
                if S >= 2:
                    gpsimd.wait_ge(s_pe, 4 * S - 4)
                rows = src[G * SC * 128:(G + 1) * SC * 128, :]
                gpsimd.dma_start(
                    cf_sb[:, (S % 2) * SC * W:(S % 2 + 1) * SC * W],
                    rows.rearrange("(a p) c -> p a c", p=128),
                ).then_inc(s_dma, 16)
            # prefetch first two inverse stream chunks
            for j in range(RSI):
                gpsimd.dma_start(
                    iv_sb[:, (j % RSI) * 4 * KF:(j % RSI + 1) * 4 * KF],
                    iv[j, :, :]).then_inc(s_dma, 16)
            # theta rows: move th4 columns (128,1) into (1,128) row tiles.
            # (PE transpose of a single column is broken on HW; DMA moves
            # freely across partitions.)
            gpsimd.wait_ge(s_dve, 12)
            for dc in range(NDC):
                gpsimd.dma_start(
                    trows[dc][:, :],
                    m8[:, dc * 8 + TOPK - 1: dc * 8 + TOPK]).then_inc(s_dma, 16)
            # remaining inverse streams interleaved with output stores
            # (program-order cycle otherwise: inv-DMA gating needs PE
            # progress -> ACT evictions -> out-DMA completions)
            for j in range(RSI, NTC + 2):
                if j < NTC:
                    gpsimd.wait_ge(s_pe, 66 + j - RSI)
                    gpsimd.dma_start(
                        iv_sb[:, (j % RSI) * 4 * KF:(j % RSI + 1) * 4 * KF],
                        iv[j, :, :]).then_inc(s_dma, 16)
                if j >= 2:
                    tcb = j - 2
                    gpsimd.wait_ge(s_act, 66 + tcb)
                    gpsimd.dma_start(
                        out[tcb * 128:(tcb + 1) * 128, :],
                        ot_sb[:, (tcb % 2) * D:(tcb % 2 + 1) * D],
                    ).then_inc(s_dma, 16)
            gpsimd.wait_ge(s_dma, 736)

        @block.tensor
        def _(tensor):
            W = 2 * KF
            # forward DFT
            for comp in range(2):
                for tcb in range(NTC):
                    g = comp * NTC + tcb
                    S = g // SC
                    tensor.wait_ge(s_dma, 32 + (S + 1) * 16)
                    if comp == 1 and tcb == 0:
                        tensor.wait_ge(s_act, 8)  # Re banks evicted
                    base = (S % 2) * SC * W + (g % SC) * W
                    xh_c = xh_sb[:, tcb * D:(tcb + 1) * D]
                    xl_c = xl_sb[:, tcb * D:(tcb + 1) * D]
                    first = (tcb == 0)
                    last = (tcb == NTC - 1)
                    for kc in range(NKC):
                        hsl = slice(base + kc * 128, base + (kc + 1) * 128)
                        lsl = slice(base + KF + kc * 128,
                                    base + KF + (kc + 1) * 128)
                        nc.tensor.matmul(banks[kc][:], cf_sb[:, hsl], xh_c,
                                         start=first, stop=False)
                        nc.tensor.matmul(banks[kc][:], cf_sb[:, hsl], xl_c,
                                         start=False, stop=False)
                        nc.tensor.matmul(banks[kc][:], cf_sb[:, lsl], xh_c,
                                         start=False, stop=False)
                        mm = nc.tensor.matmul(banks[kc][:], cf_sb[:, lsl], xl_c,
                                              start=False, stop=last)
                        if kc == NKC - 1:
                            mm.then_inc(s_pe, 1)
            # mag transposes (d-major so each dc finishes contiguously)
            tensor.wait_ge(s_pool, 2)
            for dc in range(NDC):
                for kc in range(NKC):
                    i = dc * NKC + kc
                    tensor.wait_ge(s_dve, kc + 1)
                    tensor.wait_ge(s_act, 13 + i if i >= 4 else 9 + i)
                    nc.tensor.transpose(
                        banks[i % 4][:, 0:128],
                        mag[:, kc * D + dc * 128: kc * D + (dc + 1) * 128],
                        ident[:]).then_inc(s_pe, 1)
            # ones-broadcast (fp32, exact): trow rows -> thb psum (bank5)
            tensor.wait_ge(s_dma, 16 * 16)  # theta row DMAs done
            for dc in range(NDC):
                mm = nc.tensor.matmul(pb5[:, dc * 128:(dc + 1) * 128],
                                      ones[:], trows[dc][:],
                                      start=(dc == 0), stop=(dc == NDC - 1))
                if dc == NDC - 1:
                    mm.then_inc(s_pe, 1)
            # inverse DFT
            tensor.wait_ge(s_act, 65)  # r2h/i2h casts done
            for tcb in range(NTC):
                tensor.wait_ge(
                    s_dma,
                    16 * (13 + 2 * tcb) if tcb >= RSI else 16 * (11 + tcb))
                if tcb >= 4:
                    tensor.wait_ge(s_act, 62 + tcb)  # bank evicted
                bank = banks[tcb % 4]
                sl0 = (tcb % RSI) * 4 * KF
                for kc in range(NKC):
                    dsl = slice(kc * D, (kc + 1) * D)
                    for m, (mat, coef) in enumerate(
                        ((0, r2h), (1, r2h), (2, i2h), (3, i2h))
                    ):
                        ksl = slice(sl0 + mat * KF + kc * 128,
                                    sl0 + mat * KF + (kc + 1) * 128)
                        mm = nc.tensor.matmul(
                            bank[:], iv_sb[:, ksl], coef[:, dsl],
                            start=(kc == 0 and m == 0),
                            stop=(kc == NKC - 1 and m == 3))
                    if kc == NKC - 1:
                        mm.then_inc(s_pe, 1)

        @block.scalar
        def _(scalar):
            # forward evictions; x2 scale folds the conjugate doubling
            scalar.wait_ge(s_pe, 16)
            for kc in range(NKC):
                nc.scalar.activation(r2[:, kc * D:(kc + 1) * D], banks[kc][:],
                                     AF.Copy, scale=2.0).then_inc(s_act, 1)
            scalar.wait_ge(s_pe, 32)
            for kc in range(NKC):
                nc.scalar.activation(i2[:, kc * D:(kc + 1) * D], banks[kc][:],
                                     AF.Copy, scale=2.0).then_inc(s_act, 1)
            # transpose copies
            for dc in range(NDC):
                for kc in range(NKC):
                    i = dc * NKC + kc
                    scalar.wait_ge(s_pe, 33 + i)
                    nc.scalar.activation(
                        mag_t[:, dc * KF + kc * 128: dc * KF + (kc + 1) * 128],
                        banks[i % 4][:, 0:128], AF.Copy).then_inc(s_act, 1)
            # thb copy
            scalar.wait_ge(s_pe, 65)
            nc.scalar.activation(thb[:], pb5[:], AF.Copy).then_inc(s_act, 1)
            # masked coefficient casts to bf16
            for kc in range(NKC):
                scalar.wait_ge(s_dve, 13 + kc)
                dsl = slice(kc * D, (kc + 1) * D)
                nc.scalar.activation(r2h[:, dsl], r2[:, dsl],
                                     AF.Copy).then_inc(s_act, 1)
                nc.scalar.activation(i2h[:, dsl], i2[:, dsl],
                                     AF.Copy).then_inc(s_act, 1)
            # inverse evictions
            for tcb in range(NTC):
                scalar.wait_ge(s_pe, 66 + tcb)
                if tcb >= 2:
                    # out-DMA (tcb-2) completes at 16*(18+2*(tcb-2))
                    scalar.wait_ge(s_dma, 16 * (14 + 2 * tcb))
                nc.scalar.activation(
                    ot_sb[:, (tcb % 2) * D:(tcb % 2 + 1) * D],
                    banks[tcb % 4][:], AF.Copy).then_inc(s_act, 1)

        @block.vector
        def _(vector):
            # magnitudes
            for kc in range(NKC):
                vector.wait_ge(s_act, 9 + kc)
                dsl = slice(kc * D, (kc + 1) * D)
                nc.vector.tensor_tensor(mag[:, dsl], r2[:, dsl], r2[:, dsl],
                                        ALU.mult)
                nc.vector.tensor_tensor(sqt[:], i2[:, dsl], i2[:, dsl],
                                        ALU.mult)
                nc.vector.tensor_tensor(mag[:, dsl], mag[:, dsl], sqt[:],
                                        ALU.add).then_inc(s_dve, 1)
            # top-8 + 6th-largest per channel
            for dc in range(NDC):
                vector.wait_ge(s_act, 24 + dc * 8)
                nc.vector.max(out=m8[:, dc * 8:(dc + 1) * 8],
                              in_=mag_t[:, dc * KF:(dc + 1) * KF]).then_inc(s_dve, 1)
            # mask + apply (in place)
            vector.wait_ge(s_act, 49)
            for kc in range(NKC):
                dsl = slice(kc * D, (kc + 1) * D)
                nc.vector.tensor_tensor(msk[:], mag[:, dsl], thb[:], ALU.is_ge)
                nc.vector.tensor_tensor(r2[:, dsl], r2[:, dsl], msk[:],
                                        ALU.mult)
                nc.vector.tensor_tensor(i2[:, dsl], i2[:, dsl], msk[:],
                                        ALU.mult).then_inc(s_dve, 1)


# ---------------- host side ----------------

_BF = ml_dtypes.bfloat16


def _split_hilo(a32):
    hi = a32.astype(_BF)
    lo = (a32 - hi.astype(np.float32)).astype(_BF)
    return hi, lo


def _make_constants():
    t = np.arange(T, dtype=np.float64)[:, None]
    k = np.arange(1, KF + 1, dtype=np.float64)[None, :]
    ang = 2.0 * np.pi * t * k / T
    C = np.cos(ang)
    S = -np.sin(ang)
    C[:, KF - 1] = 0.0
    S[:, KF - 1] = 0.0
    C32 = C.astype(np.float32)
    S32 = S.astype(np.float32)
    cfh, cfl = _split_hilo(C32)
    sfh, sfl = _split_hilo(S32)
    cf = np.ascontiguousarray(np.concatenate([cfh, cfl], axis=1))
    sfc = np.ascontiguousarray(np.concatenate([sfh, sfl], axis=1))

    def blocks(m32):
        M = np.ascontiguousarray(m32.T)                        # (KF, T)
        blk = M.reshape(NKC, 128, NTC, 128)                    # (kc, p, tc, u)
        blk = np.ascontiguousarray(blk.transpose(2, 1, 0, 3))  # (tc, p, kc, u)
        return blk.reshape(NTC, 128, KF)

    Cb = blocks(C32)
    Sb = blocks(S32)
    cih, cil = _split_hilo(Cb)
    sih, sil = _split_hilo(Sb)
    ivc = np.ascontiguousarray(
        np.concatenate([cih, cil, sih, sil], axis=2))          # (NTC,128,4KF)
    return dict(cf=cf, sf=sfc, iv=ivc)


_CONSTS = None
LAST_EXEC_NS = None
LAST_RES = None
TRACE = False


def kernel(input_tensor: np.ndarray) -> np.ndarray:
    from concourse.bass_utils import run_bass_kernel_spmd

    global _CONSTS
    if _CONSTS is None:
        _CONSTS = _make_constants()

    x = np.asarray(input_tensor, dtype=np.float32)
    B = x.shape[0]
    assert x.shape == (B, T, D)

    nc = bass.Bass("TRN2", target_bir_lowering=False)
    build_kernel(nc)

    in_maps = []
    for b in range(B):
        xh_np, xl_np = _split_hilo(x[b])
        in_maps.append({"xh": xh_np, "xl": xl_np, **_CONSTS})

    global LAST_EXEC_NS, LAST_RES
    res = run_bass_kernel_spmd(nc, in_maps, core_ids=list(range(B)), trace=TRACE)
    LAST_EXEC_NS = res.exec_time_ns
    LAST_RES = res
    return np.stack([res.results[b]["out"] for b in range(B)], axis=0)


if __name__ == "__main__":
    rng = np.random.default_rng(0)
    x = rng.standard_normal((8, T, D), dtype=np.float32)
    y = kernel(input_tensor=x)
    print("out", y.shape, y.dtype)



# revision 10
# speedup vs baseline: 1.2952x; 1.2952x over previous
"""FourierLayer TRN2 kernel: per-core DFT -> top-6 mask -> sparse inverse DFT.

Contract: kernel(input_tensor=(8,2048,512) f32) -> (8,2048,512) f32.
Each of the 8 NeuronCores processes one batch element (data-parallel over
batch; no cross-core communication).

Per-core pipeline (all big matmuls bf16 hi/lo split, fp32 PSUM accumulation):
  Re[k,d] = sum_t cos(2pi k t/T) x[t,d]      k = 1..1024 (k=1024 zero-padded)
  Im[k,d] = sum_t -sin(2pi k t/T) x[t,d]
  R2 = 2*Re, I2 = 2*Im  (x2 folded into PSUM eviction scale)
  mag = R2^2 + I2^2  (ordering-equivalent to |rfft|)
  theta[d] = 6th largest mag over k  (PE transpose -> vector.max top-8)
  mask = mag >= theta  (exactly the top-6 per channel; ties a.s. absent)
  out[t,d] = sum_k mask*R2*cos(2pi k t/T) + mask*I2*(-sin(2pi k t/T))

Raw bass with manual semaphores: this toolchain's walrus rejects instructions
carrying >2 sync commands, which rules out TileContext auto-sync (its kernel
tail drain waits on every proc lane). All DMAs go through gpsimd/SWDGE: each
128-partition transfer increments the DMA semaphore once per SDMA engine
(16 total), so a cumulative wait value is reached only when every transfer
counted in it has fully completed - cumulative thresholds are sound.
DMAs are coarsened (hi|lo matrices concatenated host-side, 4-chunk
super-loads) to amortize the ~2us per-dma_start SWDGE emission cost.
"""

from contextlib import ExitStack

import numpy as np
import ml_dtypes

import concourse.bass as bass
import concourse.mybir as mybir

BF16 = mybir.dt.bfloat16
F32 = mybir.dt.float32
AF = mybir.ActivationFunctionType
ALU = mybir.AluOpType

T = 2048          # time length
D = 512           # channels
KF = 1024         # padded frequency count (col j <-> k=j+1; col KF-1 zeroed)
NTC = T // 128    # 16 time chunks
NKC = KF // 128   # 8 freq chunks
NDC = D // 128    # 4 channel chunks
TOPK = 6
SC = 4            # forward super-chunk: tc-chunks per DMA
NSC = 2 * NTC // SC   # 8 forward super-chunks (2 components x 4)
RSI = 2           # inverse stream ring slots

# ---- semaphore schedules (cumulative values, 16 per DMA) ----
# s_dma units of 16, gpsimd program order: x 2 (->2); fwd supers 8 (S done at
#   3+S; ->10); inv0,inv1 (11,12); theta row DMAs (13..16); then interleaved
#   inv_j (j>=2) at 13+2j and out_i at 18+2i; total 46 units = 736.
# s_pe:  fwd groups (comp,tc) 1..32; mag transposes 33..64; ones-bcast 65;
#        inverse groups 66..81
# s_act: Re evicts 1..8; Im evicts 9..16; transpose copies 17..48; thb 49;
#        r2h/i2h casts 50..65; out evicts 66..81
# s_dve: mag 1..8; max8+th4 9..12; mask groups 13..20
# s_pool: ones 1; identity 2


def build_kernel(nc: bass.Bass):
    xh = nc.dram_tensor("xh", (T, D), BF16, kind="ExternalInput")
    xl = nc.dram_tensor("xl", (T, D), BF16, kind="ExternalInput")
    # forward DFT matrices, hi|lo concatenated along columns: [t, 2*KF]
    cf = nc.dram_tensor("cf", (T, 2 * KF), BF16, kind="ExternalInput")
    sf = nc.dram_tensor("sf", (T, 2 * KF), BF16, kind="ExternalInput")
    # inverse blocks per t-chunk: [tc, p, 2*KF] = [ci | si] (single bf16;
    # inverse precision only enters the output amplitude, not selection),
    # where ci[tc, p, kc*128+u] = C[kc*128+p, tc*128+u] etc.
    iv = nc.dram_tensor("iv", (NTC, 128, 2 * KF), BF16, kind="ExternalInput")
    out = nc.dram_tensor("out", (T, D), F32, kind="ExternalOutput")

    with ExitStack() as ctx:
        def sb(name, shape, dtype):
            return ctx.enter_context(nc.sbuf_tensor(name, shape, dtype))

        xh_sb = sb("xh_sb", [128, NTC * D], BF16)
        xl_sb = sb("xl_sb", [128, NTC * D], BF16)
        # forward ring: 2 super-slots x (SC tc-chunks x 2KF hi|lo cols)
        cf_sb = sb("cf_sb", [128, 2 * SC * 2 * KF], BF16)
        # inverse ring: RSI slots x 2*KF
        iv_sb = sb("iv_sb", [128, RSI * 2 * KF], BF16)
        r2 = sb("r2", [128, NKC * D], F32)
        i2 = sb("i2", [128, NKC * D], F32)
        r2h = sb("r2h", [128, NKC * D], BF16)
        i2h = sb("i2h", [128, NKC * D], BF16)
        mag = sb("mag", [128, NKC * D], F32)
        mag_t = sb("mag_t", [128, NDC * KF], F32)
        m8 = sb("m8", [128, NDC * 8], F32)
        trows = [sb(f"trow{i}", [1, 128], F32) for i in range(NDC)]
        thb = sb("thb", [128, D], F32)
        ones = sb("ones", [1, 128], F32)
        ident = sb("ident", [128, 128], F32)
        msk = sb("msk", [128, D], F32)
        sqt = sb("sqt", [128, D], F32)
        ot_sb = sb("ot_sb", [128, 2 * D], F32)
        banks = [ctx.enter_context(nc.psum_tensor(f"pb{i}", [128, D], F32))
                 for i in range(8)]
        pb5 = banks[5]
        s_dma = ctx.enter_context(nc.semaphore())
        s_pe = ctx.enter_context(nc.semaphore())
        s_act = ctx.enter_context(nc.semaphore())
        s_dve = ctx.enter_context(nc.semaphore())
        s_pool = ctx.enter_context(nc.semaphore())
        block = ctx.enter_context(nc.Block())

        @block.gpsimd
        def _(gpsimd):
            # constants
            gpsimd.memset(ones[:], 1.0).then_inc(s_pool, 1)
            gpsimd.memset(ident[:], 0.0)
            gpsimd.drain()
            nc.gpsimd.affine_select(
                out=ident[:], in_=ident[:],
                compare_op=ALU.not_equal, fill=1.0, base=0,
                pattern=[[-1, 128]], channel_multiplier=1,
            ).then_inc(s_pool, 1)
            # x loads: single DMA each, (tc p) d -> p (tc d)
            gpsimd.dma_start(
                xh_sb[:, :],
                xh[:].rearrange("(a p) d -> p a d", p=128)).then_inc(s_dma, 16)
            gpsimd.dma_start(
                xl_sb[:, :],
                xl[:].rearrange("(a p) d -> p a d", p=128)).then_inc(s_dma, 16)
            # forward super-chunks: S = comp*2+G over (cf, sf)
            W = 2 * KF
            for S in range(NSC):
                comp, G = divmod(S, NSC // 2)
                src = (cf, sf)[comp]
                if S >= 2:
                    gpsimd.wait_ge(s_pe, 4 * S - 4)
                rows = src[G * SC * 128:(G + 1) * SC * 128, :]
                gpsimd.dma_start(
                    cf_sb[:, (S % 2) * SC * W:(S % 2 + 1) * SC * W],
                    rows.rearrange("(a p) c -> p a c", p=128),
                ).then_inc(s_dma, 16)
            # prefetch first two inverse stream chunks
            for j in range(RSI):
                gpsimd.dma_start(
                    iv_sb[:, (j % RSI) * 2 * KF:(j % RSI + 1) * 2 * KF],
                    iv[j, :, :]).then_inc(s_dma, 16)
            # theta rows: move th4 columns (128,1) into (1,128) row tiles.
            # (PE transpose of a single column is broken on HW; DMA moves
            # freely across partitions.)
            gpsimd.wait_ge(s_dve, 12)
            for dc in range(NDC):
                gpsimd.dma_start(
                    trows[dc][:, :],
                    m8[:, dc * 8 + TOPK - 1: dc * 8 + TOPK]).then_inc(s_dma, 16)
            # remaining inverse streams interleaved with output stores
            # (program-order cycle otherwise: inv-DMA gating needs PE
            # progress -> ACT evictions -> out-DMA completions)
            for j in range(RSI, NTC + 2):
                if j < NTC:
                    gpsimd.wait_ge(s_pe, 66 + j - RSI)
                    gpsimd.dma_start(
                        iv_sb[:, (j % RSI) * 2 * KF:(j % RSI + 1) * 2 * KF],
                        iv[j, :, :]).then_inc(s_dma, 16)
                if j >= 2:
                    tcb = j - 2
                    gpsimd.wait_ge(s_act, 66 + tcb)
                    gpsimd.dma_start(
                        out[tcb * 128:(tcb + 1) * 128, :],
                        ot_sb[:, (tcb % 2) * D:(tcb % 2 + 1) * D],
                    ).then_inc(s_dma, 16)
            gpsimd.wait_ge(s_dma, 736)

        @block.tensor
        def _(tensor):
            W = 2 * KF
            # forward DFT
            for comp in range(2):
                for tcb in range(NTC):
                    g = comp * NTC + tcb
                    S = g // SC
                    tensor.wait_ge(s_dma, 32 + (S + 1) * 16)
                    if comp == 1 and tcb == 0:
                        tensor.wait_ge(s_act, 8)  # Re banks evicted
                    base = (S % 2) * SC * W + (g % SC) * W
                    xh_c = xh_sb[:, tcb * D:(tcb + 1) * D]
                    xl_c = xl_sb[:, tcb * D:(tcb + 1) * D]
                    first = (tcb == 0)
                    last = (tcb == NTC - 1)
                    for kc in range(NKC):
                        hsl = slice(base + kc * 128, base + (kc + 1) * 128)
                        lsl = slice(base + KF + kc * 128,
                                    base + KF + (kc + 1) * 128)
                        # 3-matmul hi/lo product; the lo*lo term is below
                        # the top-6 selection noise floor
                        nc.tensor.matmul(banks[kc][:], cf_sb[:, hsl], xh_c,
                                         start=first, stop=False)
                        nc.tensor.matmul(banks[kc][:], cf_sb[:, hsl], xl_c,
                                         start=False, stop=False)
                        mm = nc.tensor.matmul(banks[kc][:], cf_sb[:, lsl], xh_c,
                                              start=False, stop=last)
                        if kc == NKC - 1:
                            mm.then_inc(s_pe, 1)
            # mag transposes (d-major so each dc finishes contiguously)
            tensor.wait_ge(s_pool, 2)
            for dc in range(NDC):
                for kc in range(NKC):
                    i = dc * NKC + kc
                    tensor.wait_ge(s_dve, kc + 1)
                    tensor.wait_ge(s_act, 13 + i if i >= 4 else 9 + i)
                    nc.tensor.transpose(
                        banks[i % 4][:, 0:128],
                        mag[:, kc * D + dc * 128: kc * D + (dc + 1) * 128],
                        ident[:]).then_inc(s_pe, 1)
            # ones-broadcast (fp32, exact): trow rows -> thb psum (bank5)
            tensor.wait_ge(s_dma, 16 * 16)  # theta row DMAs done
            for dc in range(NDC):
                mm = nc.tensor.matmul(pb5[:, dc * 128:(dc + 1) * 128],
                                      ones[:], trows[dc][:],
                                      start=(dc == 0), stop=(dc == NDC - 1))
                if dc == NDC - 1:
                    mm.then_inc(s_pe, 1)
            # inverse DFT
            tensor.wait_ge(s_act, 65)  # r2h/i2h casts done
            for tcb in range(NTC):
                tensor.wait_ge(
                    s_dma,
                    16 * (13 + 2 * tcb) if tcb >= RSI else 16 * (11 + tcb))
                if tcb >= 4:
                    tensor.wait_ge(s_act, 62 + tcb)  # bank evicted
                bank = banks[tcb % 4]
                sl0 = (tcb % RSI) * 2 * KF
                for kc in range(NKC):
                    dsl = slice(kc * D, (kc + 1) * D)
                    for m, coef in ((0, r2h), (1, i2h)):
                        ksl = slice(sl0 + m * KF + kc * 128,
                                    sl0 + m * KF + (kc + 1) * 128)
                        mm = nc.tensor.matmul(
                            bank[:], iv_sb[:, ksl], coef[:, dsl],
                            start=(kc == 0 and m == 0),
                            stop=(kc == NKC - 1 and m == 1))
                    if kc == NKC - 1:
                        mm.then_inc(s_pe, 1)

        @block.scalar
        def _(scalar):
            # forward evictions; x2 scale folds the conjugate doubling
            scalar.wait_ge(s_pe, 16)
            for kc in range(NKC):
                nc.scalar.activation(r2[:, kc * D:(kc + 1) * D], banks[kc][:],
                                     AF.Copy, scale=2.0).then_inc(s_act, 1)
            scalar.wait_ge(s_pe, 32)
            for kc in range(NKC):
                nc.scalar.activation(i2[:, kc * D:(kc + 1) * D], banks[kc][:],
                                     AF.Copy, scale=2.0).then_inc(s_act, 1)
            # transpose copies
            for dc in range(NDC):
                for kc in range(NKC):
                    i = dc * NKC + kc
                    scalar.wait_ge(s_pe, 33 + i)
                    nc.scalar.activation(
                        mag_t[:, dc * KF + kc * 128: dc * KF + (kc + 1) * 128],
                        banks[i % 4][:, 0:128], AF.Copy).then_inc(s_act, 1)
            # thb copy
            scalar.wait_ge(s_pe, 65)
            nc.scalar.activation(thb[:], pb5[:], AF.Copy).then_inc(s_act, 1)
            # masked coefficient casts to bf16
            for kc in range(NKC):
                scalar.wait_ge(s_dve, 13 + kc)
                dsl = slice(kc * D, (kc + 1) * D)
                nc.scalar.activation(r2h[:, dsl], r2[:, dsl],
                                     AF.Copy).then_inc(s_act, 1)
                nc.scalar.activation(i2h[:, dsl], i2[:, dsl],
                                     AF.Copy).then_inc(s_act, 1)
            # inverse evictions
            for tcb in range(NTC):
                scalar.wait_ge(s_pe, 66 + tcb)
                if tcb >= 2:
                    # out-DMA (tcb-2) completes at 16*(18+2*(tcb-2))
                    scalar.wait_ge(s_dma, 16 * (14 + 2 * tcb))
                nc.scalar.activation(
                    ot_sb[:, (tcb % 2) * D:(tcb % 2 + 1) * D],
                    banks[tcb % 4][:], AF.Copy).then_inc(s_act, 1)

        @block.vector
        def _(vector):
            # magnitudes
            for kc in range(NKC):
                vector.wait_ge(s_act, 9 + kc)
                dsl = slice(kc * D, (kc + 1) * D)
                nc.vector.tensor_tensor(mag[:, dsl], r2[:, dsl], r2[:, dsl],
                                        ALU.mult)
                nc.vector.tensor_tensor(sqt[:], i2[:, dsl], i2[:, dsl],
                                        ALU.mult)
                nc.vector.tensor_tensor(mag[:, dsl], mag[:, dsl], sqt[:],
                                        ALU.add).then_inc(s_dve, 1)
            # top-8 + 6th-largest per channel
            for dc in range(NDC):
                vector.wait_ge(s_act, 24 + dc * 8)
                nc.vector.max(out=m8[:, dc * 8:(dc + 1) * 8],
                              in_=mag_t[:, dc * KF:(dc + 1) * KF]).then_inc(s_dve, 1)
            # mask + apply (in place)
            vector.wait_ge(s_act, 49)
            for kc in range(NKC):
                dsl = slice(kc * D, (kc + 1) * D)
                nc.vector.tensor_tensor(msk[:], mag[:, dsl], thb[:], ALU.is_ge)
                nc.vector.tensor_tensor(r2[:, dsl], r2[:, dsl], msk[:],
                                        ALU.mult)
                nc.vector.tensor_tensor(i2[:, dsl], i2[:, dsl], msk[:],
                                        ALU.mult).then_inc(s_dve, 1)


# ---------------- host side ----------------

_BF = ml_dtypes.bfloat16


def _split_hilo(a32):
    hi = a32.astype(_BF)
    lo = (a32 - hi.astype(np.float32)).astype(_BF)
    return hi, lo


def _make_constants():
    t = np.arange(T, dtype=np.float64)[:, None]
    k = np.arange(1, KF + 1, dtype=np.float64)[None, :]
    ang = 2.0 * np.pi * t * k / T
    C = np.cos(ang)
    S = -np.sin(ang)
    C[:, KF - 1] = 0.0
    S[:, KF - 1] = 0.0
    C32 = C.astype(np.float32)
    S32 = S.astype(np.float32)
    cfh, cfl = _split_hilo(C32)
    sfh, sfl = _split_hilo(S32)
    cf = np.ascontiguousarray(np.concatenate([cfh, cfl], axis=1))
    sfc = np.ascontiguousarray(np.concatenate([sfh, sfl], axis=1))

    def blocks(m32):
        M = np.ascontiguousarray(m32.T)                        # (KF, T)
        blk = M.reshape(NKC, 128, NTC, 128)                    # (kc, p, tc, u)
        blk = np.ascontiguousarray(blk.transpose(2, 1, 0, 3))  # (tc, p, kc, u)
        return blk.reshape(NTC, 128, KF)

    Cb = blocks(C32)
    Sb = blocks(S32)
    ivc = np.ascontiguousarray(
        np.concatenate([Cb.astype(_BF), Sb.astype(_BF)], axis=2))  # (NTC,128,2KF)
    return dict(cf=cf, sf=sfc, iv=ivc)


_CONSTS = None
LAST_EXEC_NS = None
LAST_RES = None
TRACE = False


def kernel(input_tensor: np.ndarray) -> np.ndarray:
    from concourse.bass_utils import run_bass_kernel_spmd

    global _CONSTS
    if _CONSTS is None:
        _CONSTS = _make_constants()

    x = np.asarray(input_tensor, dtype=np.float32)
    B = x.shape[0]
    assert x.shape == (B, T, D)

    nc = bass.Bass("TRN2", target_bir_lowering=False)
    build_kernel(nc)

    in_maps = []
    for b in range(B):
        xh_np, xl_np = _split_hilo(x[b])
        in_maps.append({"xh": xh_np, "xl": xl_np, **_CONSTS})

    global LAST_EXEC_NS, LAST_RES
    res = run_bass_kernel_spmd(nc, in_maps, core_ids=list(range(B)), trace=TRACE)
    LAST_EXEC_NS = res.exec_time_ns
    LAST_RES = res
    return np.stack([res.results[b]["out"] for b in range(B)], axis=0)


if __name__ == "__main__":
    rng = np.random.default_rng(0)
    x = rng.standard_normal((8, T, D), dtype=np.float32)
    y = kernel(input_tensor=x)
    print("out", y.shape, y.dtype)

